# revision 16
# baseline (speedup 1.0000x reference)
"""Trainium2 Bass kernel for ragged-sequence attention (G2/f16/split-DMA).

Per batch b:
    tq     = tanh(query[b] @ W + bias)                      [CA, H]
    scores = key[b] @ tq.T                                  [S, CA]
    alpha  = exp(scores) * (s < seq_len[b])                 [S, CA]
    out[b] = (alpha.T @ value[b]) / alpha.sum(axis=0)[:,None]

Strategy (HBM-bandwidth bound; everything serves DMA bytes):
  - Raggedness: independent 128-row sub-chunks of each valid prefix;
    numerator/denominator are additive over s, each sub yields a partial
    [CA, 768+1] (col 768 = denominator via a ones-column in the value tile).
  - Subs are spread round-robin over 8 cores, packed 2 per "group"; one
    group = two DMAs (~0.5MB keyT/tq/mask half on the SP HWDGE ring, ~0.4MB
    value half on the ACT HWDGE ring) for parallel descriptor streams.
    Identical NEFF on all cores (SPMD); dummy subs have zero tq/mask.
    Host does the tiny group-by-batch reduction and division.
  - Streams in fp16 (better mantissa than bf16 at equal bytes; inputs are
    O(1)-ranged so fp16's range is ample). BASS_ATTN_DT=bf16 / f32r
    switch the stream dtype. exp and psum accumulation stay fp32;
    partial outputs return as fp16.
  - key is pre-transposed on the host into [128, 6, 128] h-major tiles
    (4-byte dtypes have no DMA-transpose path, and the host does it for
    free); value tiles are [128, 772] s-major with ones at col 768.
"""

import os
import sys

import numpy as np

for _p in ("/opt/trn_rl_repo", "/root/.axon_site/_ro/trn_rl_repo"):
    if os.path.isdir(_p) and _p not in sys.path:
        sys.path.append(_p)

N_CORES = 8
SUB = 128        # rows per work item (= matmul contraction dim)
G = 2            # sub-chunks per group (one DMA / processing slot)
H = 768
HSUB = H // 128  # 6
CA = 32
VW = 772         # value tile width: 768 value cols + ones col @768 + pad

TQ_W = HSUB * CA              # 192 per sub
TQ_OFF = 0
MK_OFF = TQ_OFF + G * TQ_W    # 384
MK_W = G                      # 2
ID_OFF = MK_OFF + MK_W        # 386
ID_W = CA                     # 32
KT_OFF = ID_OFF + ID_W        # 418
KT_W = HSUB * SUB             # 768 per sub
VL_OFF = KT_OFF + G * KT_W    # 1954
COMB_W = VL_OFF + G * VW      # 3498

DT = os.environ.get("BASS_ATTN_DT", "f16")

_module_cache = {}
_last_in_maps = None


def _np_dt():
    if DT == "bf16":
        import ml_dtypes

        return ml_dtypes.bfloat16
    if DT == "f16":
        return np.float16
    return np.float32


def _build_module(nch, loop_r=None):
    import contextlib
    import concourse.mybir as mybir
    import concourse.tile as tile
    from concourse import bacc

    f32 = mybir.dt.float32
    f16 = mybir.dt.float16
    mmdt = {
        "bf16": mybir.dt.bfloat16,
        "f16": mybir.dt.float16,
        "f32r": mybir.dt.float32r,
    }[DT]
    AF = mybir.ActivationFunctionType

    nc = bacc.Bacc(None, target_bir_lowering=False, enable_asserts=False)
    comb_d = nc.dram_tensor("comb", [nch, 128, COMB_W], mmdt, kind="ExternalInput")
    out_d = nc.dram_tensor("outp", [nch, CA, G, VW], f16, kind="ExternalOutput")

    with tile.TileContext(nc) as tc:
        with (
            tc.tile_pool(name="big", bufs=8) as big,
            tc.tile_pool(name="work", bufs=5) as work,
            tc.tile_pool(name="ps_s", bufs=2, space="PSUM") as ps_s_pool,
            tc.tile_pool(name="ps_t", bufs=2, space="PSUM") as ps_t_pool,
            tc.tile_pool(name="ps_o", bufs=2, space="PSUM") as ps_o_pool,
            tc.For_i(0, loop_r, 1) if loop_r else contextlib.nullcontext(),
        ):
            for i in range(nch):
                ct = big.tile([128, COMB_W], mmdt, tag="comb")
                # kt/tq/mask half on the SP HWDGE ring, value half on the
                # ACT HWDGE ring: parallel descriptor streams
                nc.sync.dma_start(out=ct[:, :VL_OFF], in_=comb_d[i, :, :VL_OFF])
                nc.scalar.dma_start(out=ct[:, VL_OFF:], in_=comb_d[i, :, VL_OFF:])

                tq_v = ct[:, TQ_OFF : TQ_OFF + G * TQ_W].rearrange(
                    "p (m o c) -> p m o c", m=G, o=HSUB
                )
                mk_v = ct[:, MK_OFF : MK_OFF + MK_W]
                id_v = ct[:CA, ID_OFF : ID_OFF + ID_W]
                kt_v = ct[:, KT_OFF : KT_OFF + G * KT_W].rearrange(
                    "p (m o s) -> p m o s", m=G, o=HSUB
                )
                vl_v = ct[:, VL_OFF : VL_OFF + G * VW].rearrange(
                    "p (m w) -> p m w", m=G
                )

                # scores.T: [CA, G*SUB]; sub m -> columns [m*SUB, (m+1)*SUB)
                ps_s = ps_s_pool.tile([CA, G * SUB], f32)
                for m in range(G):
                    for ho in range(HSUB):
                        nc.tensor.matmul(
                            ps_s[:, m * SUB : (m + 1) * SUB],
                            lhsT=tq_v[:, m, ho, :],
                            rhs=kt_v[:, m, ho, :],
                            start=(ho == 0),
                            stop=(ho == HSUB - 1),
                        )

                sb_e = work.tile([CA, G * SUB], mmdt, tag="exp")
                nc.scalar.activation(out=sb_e, in_=ps_s, func=AF.Exp)

                # transpose exp(scores) to s-on-partitions for the value mm
                ps_t = ps_t_pool.tile([128, G, CA], mmdt)
                for m in range(G):
                    nc.tensor.transpose(
                        ps_t[:, m, :],
                        sb_e[:, m * SUB : (m + 1) * SUB],
                        id_v,
                    )

                al_t = work.tile([128, G, CA], mmdt, tag="alpha")
                nc.vector.tensor_tensor(
                    al_t,
                    ps_t,
                    mk_v[:, :, None].to_broadcast([128, G, CA]),
                    mybir.AluOpType.mult,
                )

                # numerator (+ denominator via ones column at 768) per sub
                ob = work.tile([CA, G, VW], f16, tag="ob")
                for m in range(G):
                    ps_o = ps_o_pool.tile([CA, VW], f32, tag="ps_o")
                    nc.tensor.matmul(
                        ps_o[:, 0:512],
                        lhsT=al_t[:, m, :],
                        rhs=vl_v[:, m, 0:512],
                        start=True,
                        stop=True,
                    )
                    nc.tensor.matmul(
                        ps_o[:, 512:VW],
                        lhsT=al_t[:, m, :],
                        rhs=vl_v[:, m, 512:VW],
                        start=True,
                        stop=True,
                    )
                    if m % 2 == 0:
                        nc.vector.tensor_copy(out=ob[:, m, :], in_=ps_o)
                    else:
                        nc.scalar.copy(out=ob[:, m, :], in_=ps_o)
                nc.sync.dma_start(out=out_d[i], in_=ob)

    nc.compile()
    return nc


def kernel(key, value, query, seq_len, W, b):
    key = np.ascontiguousarray(np.asarray(key, dtype=np.float32))
    value = np.ascontiguousarray(np.asarray(value, dtype=np.float32))
    query = np.asarray(query, dtype=np.float32)
    W = np.asarray(W, dtype=np.float32)
    bias = np.asarray(b, dtype=np.float32)
    sl = np.asarray(seq_len).astype(np.int64)

    B, S, H_ = key.shape
    assert H_ == H and S % SUB == 0

    # host: tiny projection  tq[b] = tanh(query[b] @ W + bias)  [B, CA, H]
    tq = np.tanh(query.reshape(B * query.shape[1], -1) @ W + bias)
    tq = tq.reshape(B, query.shape[1], H).astype(np.float32)
    npdt = _np_dt()
    tqT_p = {
        bi: np.ascontiguousarray(tq[bi].T.reshape(HSUB, 128, CA)).astype(npdt)
        for bi in range(B)
    }

    # work list: 128-row sub-chunks over valid prefixes
    subs = []  # (batch, s0, nvalid)
    for bi in range(B):
        L = int(sl[bi])
        L = max(1, min(L, S))
        for s0 in range(0, L, SUB):
            subs.append((bi, s0, min(SUB, L - s0)))
    total = len(subs)
    per_core = -(-total // N_CORES)
    nch = -(-per_core // G)

    comb = np.zeros((N_CORES, nch, 128, COMB_W), npdt)
    comb[:, :, :CA, ID_OFF : ID_OFF + ID_W] = np.eye(CA, dtype=np.float32)
    slot_map = [[] for _ in range(N_CORES)]  # per core: list of (slot, m, batch)

    for idx, (bi, s0, nval) in enumerate(subs):
        c = idx % N_CORES
        k = idx // N_CORES
        j, m = k // G, k % G
        row = comb[c, j]
        row[:, TQ_OFF + m * TQ_W : TQ_OFF + (m + 1) * TQ_W] = (
            tqT_p[bi].transpose(1, 0, 2).reshape(128, TQ_W)
        )
        mcol = np.zeros(128, np.float32)
        mcol[:nval] = 1.0
        row[:, MK_OFF + m] = mcol
        kc = key[bi, s0 : s0 + SUB]  # [SUB, H]
        row[:, KT_OFF + m * KT_W : KT_OFF + (m + 1) * KT_W] = (
            kc.T.reshape(HSUB, 128, SUB).transpose(1, 0, 2).reshape(128, KT_W)
        )
        vt = row[:, VL_OFF + m * VW : VL_OFF + (m + 1) * VW]
        vt[:, :H] = value[bi, s0 : s0 + SUB]
        vt[:, H] = 1.0
        slot_map[c].append((j, m, bi))

    if nch not in _module_cache:
        _module_cache[nch] = _build_module(nch)
    nc = _module_cache[nch]

    from concourse.bass_utils import run_bass_kernel_spmd

    in_maps = [{"comb": comb[c]} for c in range(N_CORES)]
    global _last_in_maps
    _last_in_maps = in_maps
    trace = os.environ.get("BASS_KERNEL_TRACE") == "1"
    kwargs = {}
    if trace:
        kwargs = dict(trace=True, trace_cores=list(range(N_CORES)))
    res = run_bass_kernel_spmd(nc, in_maps, core_ids=list(range(N_CORES)), **kwargs)
    if trace and res.exec_time_ns is not None:
        print(f"HW exec time: {res.exec_time_ns} ns")
        print(f"HW exec time mean: {res.mean_exec_time_ns} ns")

    num = np.zeros((B, CA, H), np.float64)
    den = np.zeros((B, CA), np.float64)
    for c in range(N_CORES):
        part = res.results[c]["outp"]  # [nch, CA, G, VW]
        for j, m, bi in slot_map[c]:
            num[bi] += part[j, :, m, :H]
            den[bi] += part[j, :, m, H]
    out = (num / den[:, :, None]).astype(np.float32)
    return out


# revision 20
# speedup vs baseline: 1.0265x; 1.0265x over previous
"""Trainium2 Bass kernel for ragged-sequence attention (G2/f16/split-DMA).

Per batch b:
    tq     = tanh(query[b] @ W + bias)                      [CA, H]
    scores = key[b] @ tq.T                                  [S, CA]
    alpha  = exp(scores) * (s < seq_len[b])                 [S, CA]
    out[b] = (alpha.T @ value[b]) / alpha.sum(axis=0)[:,None]

Strategy (HBM-bandwidth bound; everything serves DMA bytes):
  - Raggedness: independent 128-row sub-chunks of each valid prefix;
    numerator/denominator are additive over s, each sub yields a partial
    [CA, 768+1] (col 768 = denominator via a ones-column in the value tile).
  - Subs are spread round-robin over 8 cores, packed 2 per "group"; one
    group = two DMAs (~0.5MB keyT/tq/mask half on the SP HWDGE ring, ~0.4MB
    value half on the ACT HWDGE ring) for parallel descriptor streams.
    Identical NEFF on all cores (SPMD); dummy subs have zero tq/mask.
    Host does the tiny group-by-batch reduction and division.
  - Streams in fp16 (better mantissa than bf16 at equal bytes; inputs are
    O(1)-ranged so fp16's range is ample). BASS_ATTN_DT=bf16 / f32r
    switch the stream dtype. exp and psum accumulation stay fp32;
    partial outputs return as fp16.
  - key is pre-transposed on the host into [128, 6, 128] h-major tiles
    (4-byte dtypes have no DMA-transpose path, and the host does it for
    free); value tiles are [128, 772] s-major with ones at col 768.
"""

import os
import sys

import numpy as np

for _p in ("/opt/trn_rl_repo", "/root/.axon_site/_ro/trn_rl_repo"):
    if os.path.isdir(_p) and _p not in sys.path:
        sys.path.append(_p)

N_CORES = 8
SUB = 128        # rows per work item (= matmul contraction dim)
G = 3            # sub-chunks per group (one DMA / processing slot)
H = 768
HSUB = H // 128  # 6
CA = 32
VW = 772         # value tile width: 768 value cols + ones col @768 + pad
NQ = VW // 4     # 193: value matmul runs as 4 PE col-tiles -> one psum bank

TQ_W = HSUB * CA              # 192 per sub
TQ_OFF = 0
MK_OFF = TQ_OFF + G * TQ_W    # 384
MK_W = G                      # 2
ID_OFF = MK_OFF + MK_W        # 386
ID_W = CA                     # 32
KT_OFF = ID_OFF + ID_W        # 418
KT_W = HSUB * SUB             # 768 per sub
VL_OFF = KT_OFF + G * KT_W    # 1954
COMB_W = VL_OFF + G * VW      # 3498

DT = os.environ.get("BASS_ATTN_DT", "f16")

_module_cache = {}
_last_in_maps = None


def _np_dt():
    if DT == "bf16":
        import ml_dtypes

        return ml_dtypes.bfloat16
    if DT == "f16":
        return np.float16
    return np.float32


def _build_module(nch, loop_r=None):
    import contextlib
    import concourse.mybir as mybir
    import concourse.tile as tile
    from concourse import bacc

    f32 = mybir.dt.float32
    f16 = mybir.dt.float16
    mmdt = {
        "bf16": mybir.dt.bfloat16,
        "f16": mybir.dt.float16,
        "f32r": mybir.dt.float32r,
    }[DT]
    AF = mybir.ActivationFunctionType

    nc = bacc.Bacc(None, target_bir_lowering=False, enable_asserts=False)
    comb_d = nc.dram_tensor("comb", [nch, 128, COMB_W], mmdt, kind="ExternalInput")
    out_d = nc.dram_tensor("outp", [nch, 128, G, NQ], f16, kind="ExternalOutput")

    with tile.TileContext(nc) as tc:
        with (
            tc.tile_pool(name="big", bufs=8) as big,
            tc.tile_pool(name="work", bufs=5) as work,
            tc.tile_pool(name="ps_s", bufs=2, space="PSUM") as ps_s_pool,
            tc.tile_pool(name="ps_t", bufs=2, space="PSUM") as ps_t_pool,
            tc.tile_pool(name="ps_o", bufs=3, space="PSUM") as ps_o_pool,
            tc.For_i(0, loop_r, 1) if loop_r else contextlib.nullcontext(),
        ):
            for i in range(nch):
                ct = big.tile([128, COMB_W], mmdt, tag="comb")
                # kt/tq/mask half on the SP HWDGE ring, value half on the
                # ACT HWDGE ring: parallel descriptor streams
                nc.sync.dma_start(out=ct[:, :VL_OFF], in_=comb_d[i, :, :VL_OFF])
                nc.scalar.dma_start(out=ct[:, VL_OFF:], in_=comb_d[i, :, VL_OFF:])

                tq_v = ct[:, TQ_OFF : TQ_OFF + G * TQ_W].rearrange(
                    "p (m o c) -> p m o c", m=G, o=HSUB
                )
                mk_v = ct[:, MK_OFF : MK_OFF + MK_W]
                id_v = ct[:CA, ID_OFF : ID_OFF + ID_W]
                kt_v = ct[:, KT_OFF : KT_OFF + G * KT_W].rearrange(
                    "p (m o s) -> p m o s", m=G, o=HSUB
                )
                vl_v = ct[:, VL_OFF : VL_OFF + G * VW].rearrange(
                    "p (m w) -> p m w", m=G
                )

                # scores.T: [CA, G*SUB]; sub m -> columns [m*SUB, (m+1)*SUB)
                ps_s = ps_s_pool.tile([CA, G * SUB], f32)
                for m in range(G):
                    for ho in range(HSUB):
                        nc.tensor.matmul(
                            ps_s[:, m * SUB : (m + 1) * SUB],
                            lhsT=tq_v[:, m, ho, :],
                            rhs=kt_v[:, m, ho, :],
                            start=(ho == 0),
                            stop=(ho == HSUB - 1),
                        )

                sb_e = work.tile([CA, G * SUB], mmdt, tag="exp")
                nc.scalar.activation(out=sb_e, in_=ps_s, func=AF.Exp)

                # transpose exp(scores) to s-on-partitions for the value mm
                ps_t = ps_t_pool.tile([128, G, CA], mmdt)
                for m in range(G):
                    nc.tensor.transpose(
                        ps_t[:, m, :],
                        sb_e[:, m * SUB : (m + 1) * SUB],
                        id_v,
                    )

                al_t = work.tile([128, G, CA], mmdt, tag="alpha")
                nc.vector.tensor_tensor(
                    al_t,
                    ps_t,
                    mk_v[:, :, None].to_broadcast([128, G, CA]),
                    mybir.AluOpType.mult,
                )

                # numerator (+ denominator via ones column at 768) per sub:
                # 4 concurrent PE col-tiles land the [CA, VW] output as
                # [128, NQ] in ONE psum bank, so the PSUM->SBUF copy uses
                # all 128 lanes (4x fewer cycles than a [CA, VW] copy)
                ob = work.tile([128, G, NQ], f16, tag="ob")
                for m in range(G):
                    ps_o = ps_o_pool.tile([128, NQ], f32, tag="ps_o")
                    for j in range(4):
                        nc.tensor.matmul(
                            ps_o[32 * j : 32 * (j + 1), :],
                            lhsT=al_t[:, m, :],
                            rhs=vl_v[:, m, NQ * j : NQ * (j + 1)],
                            start=True,
                            stop=True,
                            tile_position=(0, 32 * j),
                        )
                    if m % 2 == 0:
                        nc.vector.tensor_copy(out=ob[:, m, :], in_=ps_o)
                    else:
                        nc.scalar.copy(out=ob[:, m, :], in_=ps_o)
                nc.sync.dma_start(out=out_d[i], in_=ob)

    nc.compile()
    return nc


def kernel(key, value, query, seq_len, W, b):
    key = np.ascontiguousarray(np.asarray(key, dtype=np.float32))
    value = np.ascontiguousarray(np.asarray(value, dtype=np.float32))
    query = np.asarray(query, dtype=np.float32)
    W = np.asarray(W, dtype=np.float32)
    bias = np.asarray(b, dtype=np.float32)
    sl = np.asarray(seq_len).astype(np.int64)

    B, S, H_ = key.shape
    assert H_ == H and S % SUB == 0

    # host: tiny projection  tq[b] = tanh(query[b] @ W + bias)  [B, CA, H]
    tq = np.tanh(query.reshape(B * query.shape[1], -1) @ W + bias)
    tq = tq.reshape(B, query.shape[1], H).astype(np.float32)
    npdt = _np_dt()
    tqT_p = {
        bi: np.ascontiguousarray(tq[bi].T.reshape(HSUB, 128, CA)).astype(npdt)
        for bi in range(B)
    }

    # work list: 128-row sub-chunks over valid prefixes
    subs = []  # (batch, s0, nvalid)
    for bi in range(B):
        L = int(sl[bi])
        L = max(1, min(L, S))
        for s0 in range(0, L, SUB):
            subs.append((bi, s0, min(SUB, L - s0)))
    total = len(subs)
    per_core = -(-total // N_CORES)
    nch = -(-per_core // G)

    comb = np.zeros((N_CORES, nch, 128, COMB_W), npdt)
    comb[:, :, :CA, ID_OFF : ID_OFF + ID_W] = np.eye(CA, dtype=np.float32)
    slot_map = [[] for _ in range(N_CORES)]  # per core: list of (slot, m, batch)

    for idx, (bi, s0, nval) in enumerate(subs):
        c = idx % N_CORES
        k = idx // N_CORES
        j, m = k // G, k % G
        row = comb[c, j]
        row[:, TQ_OFF + m * TQ_W : TQ_OFF + (m + 1) * TQ_W] = (
            tqT_p[bi].transpose(1, 0, 2).reshape(128, TQ_W)
        )
        mcol = np.zeros(128, np.float32)
        mcol[:nval] = 1.0
        row[:, MK_OFF + m] = mcol
        kc = key[bi, s0 : s0 + SUB]  # [SUB, H]
        row[:, KT_OFF + m * KT_W : KT_OFF + (m + 1) * KT_W] = (
            kc.T.reshape(HSUB, 128, SUB).transpose(1, 0, 2).reshape(128, KT_W)
        )
        vt = row[:, VL_OFF + m * VW : VL_OFF + (m + 1) * VW]
        vt[:, :H] = value[bi, s0 : s0 + SUB]
        vt[:, H] = 1.0
        slot_map[c].append((j, m, bi))

    if nch not in _module_cache:
        _module_cache[nch] = _build_module(nch)
    nc = _module_cache[nch]

    from concourse.bass_utils import run_bass_kernel_spmd

    in_maps = [{"comb": comb[c]} for c in range(N_CORES)]
    global _last_in_maps
    _last_in_maps = in_maps
    trace = os.environ.get("BASS_KERNEL_TRACE") == "1"
    kwargs = {}
    if trace:
        kwargs = dict(trace=True, trace_cores=list(range(N_CORES)))
    res = run_bass_kernel_spmd(nc, in_maps, core_ids=list(range(N_CORES)), **kwargs)
    if trace and res.exec_time_ns is not None:
        print(f"HW exec time: {res.exec_time_ns} ns")
        print(f"HW exec time mean: {res.mean_exec_time_ns} ns")

    num = np.zeros((B, CA, H), np.float64)
    den = np.zeros((B, CA), np.float64)
    for c in range(N_CORES):
        part = res.results[c]["outp"]  # [nch, 128, G, NQ] col-tiled quarters
        for j, m, bi in slot_map[c]:
            blk = part[j, :, m, :].astype(np.float64).reshape(4, CA, NQ)
            full = np.concatenate(list(blk), axis=1)
            num[bi] += full[:, :H]
            den[bi] += full[:, H]
    out = (num / den[:, :, None]).astype(np.float32)
    return out
